# revision 1
# baseline (speedup 1.0000x reference)
"""GAT 2-layer distributed Bass kernel for TRN2 (8 cores) — v2.

Changes vs baseline:
  - single merged 768B full-row gather per view (was h 512B + a 256B +
    2x dst-a 256B = 1280B/edge in 6 gathers) -> 768B/edge in 2 gathers
  - dst attention term via per-tile adst + one-hot-transpose matmul on PE
    (no per-edge dst gathers)
  - layer-2 dense (D2) fused into the edge-1 epilogue per tile; AllGather
    chunked over 7 tile-groups so it overlaps edge-phase-1 compute
  - T1 writes batched 14 groups/DMA (was 1 DMA per 128-row group)
  - idx/dstloc loads hoisted to one DMA per layer
  - batched out2 writes

Table layout per node-slot row (768B = 384 bf16, RB):
  [asrc 8 | adst 8 | h 256 | pad 112] (all bf16)
T1 = layer-1 table (written locally by replicated dense phase)
T2 = layer-2 table (chunked AllGather of per-shard cc chunks)

Slots: NSLOT = 8 * NTILE * 128. Node->slot permutation balances edge counts
per (core, tile). Slot 0 and slot VB are zero dummies (gather-pad targets).
Views for int16 gather indices: A = rows [0, VA), B = rows [VB, NSLOT),
VA = 32768 (or NSLOT/2 for mini), VB = NSLOT - VA.
"""
import dataclasses
import numpy as np


@dataclasses.dataclass
class Cfg:
    ncores: int = 8
    ntile: int = 49          # dst tiles per core
    nchunk: int = 9          # 128-edge chunks per tile
    na: int = 512            # A-view edge slots per tile (chunks 0..na/128)
    nb: int = 640            # B-view edge slots per tile
    n: int = 50000           # real nodes
    e: int = 400000
    fin: int = 128
    h: int = 8
    c: int = 32              # layer-1 head dim (h*c = 256)
    out: int = 32            # layer-2 head dim
    bt: int = 4              # tiles per gather batch
    use_collective: bool = True
    xbatch: int = 56         # dense node-tiles per x-stream DMA
    wg: int = 28             # T1 row-groups per write DMA
    ccb: int = 7             # tiles per AllGather chunk

    @property
    def shslots(self):
        return self.ntile * 128

    @property
    def nslot(self):
        return self.ncores * self.shslots

    @property
    def va(self):
        return min(32256, self.nslot)

    @property
    def vb(self):
        return self.nslot - self.va

    @property
    def d1(self):
        return self.h * self.c      # 256

    @property
    def rec(self):
        return 16 + self.d1 // 2 + 48    # 192 f32 per row (768B, stride %256)


def host_prep(cfg: Cfg, x, edge_index, W1, as1, ad1, b1, W2, as2, ad2, b2):
    N, E = cfg.n, cfg.e
    NC, NT, SH = cfg.ncores, cfg.ntile, cfg.shslots
    CCB = cfg.ccb
    src = np.asarray(edge_index[0], dtype=np.int64)
    dst = np.asarray(edge_index[1], dtype=np.int64)
    deg = np.bincount(dst, minlength=N)

    def rowid(k, t, p):
        # chunk-major table row order so each chunked AllGather output
        # (ranks x CCB tiles) is one contiguous block of T2
        return ((t // CCB) * (NC * CCB * 128) + k * (CCB * 128)
                + (t % CCB) * 128 + p)

    # ---- assign nodes to (core, tile, slot), balancing edge counts ----
    order = np.argsort(-deg, kind="stable")
    core_load = np.zeros(NC, dtype=np.int64)
    core_cnt = np.zeros(NC, dtype=np.int64)
    cap_core = N // NC
    node_core = np.empty(N, dtype=np.int64)
    for nd in order:
        k = np.argmin(np.where(core_cnt < cap_core, core_load, np.iinfo(np.int64).max))
        node_core[nd] = k
        core_load[k] += deg[nd]
        core_cnt[k] += 1

    # reserved dummy slots: slot 0 and slot VB
    # reserved dummy rows (gather pad targets): table rows 0 and VB.
    # invert rowid to find their (core, tile, lane)
    rsv = {}
    for r in (0, cfg.vb):
        blk = NC * CCB * 128
        c, rr = divmod(r, blk)
        k, rr = divmod(rr, CCB * 128)
        tc, p = divmod(rr, 128)
        rsv.setdefault((k, c * CCB + tc), []).append(p)
    # node -> (core, tile, lane)
    node_k = np.empty(N, dtype=np.int64)
    node_t = np.empty(N, dtype=np.int64)
    node_p = np.empty(N, dtype=np.int64)
    slot2node = np.full(cfg.nslot, -1, dtype=np.int64)
    for k in range(NC):
        nodes_k = order[node_core[order] == k]
        tcap = np.full(NT, 128, dtype=np.int64)
        rsv_t = {t: ps for (kk, t), ps in rsv.items() if kk == k}
        for t, ps in rsv_t.items():
            tcap[t] -= len(ps)
        tload = np.zeros(NT, dtype=np.int64)
        tcnt = np.zeros(NT, dtype=np.int64)
        tmember = [[] for _ in range(NT)]
        for nd in nodes_k:
            t = np.argmin(np.where(tcnt < tcap, tload, np.iinfo(np.int64).max))
            tmember[t].append(nd)
            tload[t] += deg[nd]
            tcnt[t] += 1
        for t in range(NT):
            skip = set(rsv_t.get(t, []))
            lanes = [p for p in range(128) if p not in skip]
            for i, nd in enumerate(tmember[t]):
                p = lanes[i]
                node_k[nd], node_t[nd], node_p[nd] = k, t, p
                slot2node[rowid(k, t, p)] = nd

    assert slot2node[0] == -1 and slot2node[cfg.vb] == -1
    node_row = rowid(node_k, node_t, node_p)

    # ---- per (core, tile) edge lists with A/B split ----
    sslot = node_row[src]
    ecore = node_k[dst]
    etile = node_t[dst]
    dlane = node_p[dst]

    NA, NB, NCH = cfg.na, cfg.nb, cfg.nchunk
    assert NA + NB == NCH * 128 and NA % 128 == 0 and NB % 128 == 0

    idxA = np.zeros((NC, NT, NA), dtype=np.int64)      # src row, A view
    idxB = np.zeros((NC, NT, NB), dtype=np.int64)      # src row - VB
    idxD = np.zeros((NC, NT, NA + NB), dtype=np.int64)  # dst row (view by tile)
    dloc = np.full((NC, NT, NA + NB), -1.0, dtype=np.float32)

    # tiles in chunks fully below VA use the A view for the dst gather,
    # the rest use the B view (all rows of those chunks are >= VB)
    blk = NC * CCB * 128
    abt = min(NT, (cfg.va // blk) * CCB)
    for c in range(NT // CCB):
        lo, hi = c * blk, (c + 1) * blk
        if c * CCB < abt:
            assert hi <= cfg.va, (c, hi, cfg.va)
        else:
            assert lo >= cfg.vb, (c, lo, cfg.vb)

    for k in range(NC):
        for t in range(NT):
            sel = np.nonzero((ecore == k) & (etile == t))[0]
            ss = sslot[sel]
            inA = ss < cfg.va
            inB = ss >= cfg.vb
            flex = inA & inB
            forcedA = inA & ~inB
            forcedB = inB & ~inA
            a_list = list(np.nonzero(forcedA)[0])
            b_list = list(np.nonzero(forcedB)[0])
            for i in np.nonzero(flex)[0]:
                if NA - len(a_list) >= NB - len(b_list):
                    a_list.append(i)
                else:
                    b_list.append(i)
            if len(a_list) > NA or len(b_list) > NB:
                raise RuntimeError(
                    f"tile overflow core{k} tile{t}: {len(a_list)}/{NA} {len(b_list)}/{NB}"
                )
            doff = 0 if t < abt else cfg.vb
            for p, i in enumerate(a_list):
                e_id = sel[i]
                idxA[k, t, p] = ss[i]
                idxD[k, t, p] = node_row[dst[e_id]] - doff
                dloc[k, t, p] = dlane[e_id]
            for p, i in enumerate(b_list):
                e_id = sel[i]
                idxB[k, t, p] = ss[i] - cfg.vb
                idxD[k, t, NA + p] = node_row[dst[e_id]] - doff
                dloc[k, t, NA + p] = dlane[e_id]

    def pack16(v, width):
        # v [NC, NT, width] int -> [NC, NT, 128, width//16] int16 wrapped+replicated
        assert v.shape[-1] == width and width % 16 == 0
        r = v.reshape(NC, -1, width // 16, 16)
        r = np.transpose(r, (0, 1, 3, 2))  # [NC, NT, 16, width//16]
        r = np.tile(r, (1, 1, 8, 1)).astype(np.int16)
        return np.ascontiguousarray(r)

    pidxA = pack16(idxA, NA)
    pidxB = pack16(idxB, NB)
    pidxD = pack16(idxD, NA + NB)

    import ml_dtypes
    # dstloc [NC, NT, 128, NCH]: position p = j*128 + lane; pads -> -1
    dloc_t = np.transpose(dloc.reshape(NC, NT, NCH, 128), (0, 1, 3, 2))
    dloc_t = np.ascontiguousarray(dloc_t.astype(ml_dtypes.bfloat16))

    # validmask [NC, NT, 128, 1] indexed by (core, tile, lane)
    vm = np.zeros((NC, NT, 128, 1), dtype=np.float32)
    for k in range(NC):
        for t in range(NT):
            vm[k, t, :, 0] = slot2node[rowid(k, t, np.arange(128))] >= 0
    vm = np.ascontiguousarray(vm)

    # xT permuted (replicated across cores) [128, NSLOT] bf16;
    # column order == table row order (D1 writes row g*128+p from col g*128+p)
    xp = np.zeros((cfg.nslot, cfg.fin), dtype=np.float32)
    real = slot2node >= 0
    xp[real] = np.asarray(x, dtype=np.float32)[slot2node[real]]
    xT = np.ascontiguousarray(xp.T).astype(ml_dtypes.bfloat16)

    def chmaj(M, hdim, axis):
        # permute an (h, c)-ordered head block to (c, h) order along axis
        M = np.moveaxis(M, axis, 0)
        M = M.reshape(cfg.h, hdim, *M.shape[1:])
        M = np.swapaxes(M, 0, 1).reshape(cfg.h * hdim, *M.shape[2:])
        return np.moveaxis(M, 0, axis)

    def fuse(W, asv, adv, hdim):
        Wa = np.einsum("fhc,hc->fh", W.reshape(W.shape[0], cfg.h, hdim), asv)
        Wd = np.einsum("fhc,hc->fh", W.reshape(W.shape[0], cfg.h, hdim), adv)
        # attn columns first so the dense phase can copy [attn|h] in one op;
        # h-part columns in (c h) order so the DVE msg multiply gets
        # stride-1 last dims on every operand (2x/4x fast path)
        return np.concatenate([Wa, Wd, chmaj(W, hdim, 1)], axis=1)

    Wall1 = fuse(np.asarray(W1, np.float32), np.asarray(as1, np.float32),
                 np.asarray(ad1, np.float32), cfg.c).astype(ml_dtypes.bfloat16)
    Wall2f = fuse(chmaj(np.asarray(W2, np.float32), cfg.c, 0),
                  np.asarray(as2, np.float32),
                  np.asarray(ad2, np.float32), cfg.out)
    Wall2 = np.ascontiguousarray(
        Wall2f.reshape(2, 128, Wall2f.shape[1])).astype(ml_dtypes.bfloat16)

    b1t = np.ascontiguousarray(np.tile(
        chmaj(np.asarray(b1, np.float32), cfg.c, 0), (128, 1)))
    b2t = np.ascontiguousarray(np.tile(np.asarray(b2, np.float32), (128, 1)))
    iota = np.tile(np.arange(128, dtype=np.float32), (128, 1)).astype(ml_dtypes.bfloat16)
    ident = np.eye(128, dtype=np.float32).astype(ml_dtypes.bfloat16)

    # out2 row (k, t*128+p) -> node id (-1 for padding lanes)
    out_node = np.full((NC, SH), -1, dtype=np.int64)
    for k in range(NC):
        for t in range(NT):
            out_node[k, t * 128:(t + 1) * 128] = \
                slot2node[rowid(k, t, np.arange(128))]

    in_maps = []
    for k in range(NC):
        in_maps.append({
            "xT": xT, "Wall1": Wall1, "Wall2": Wall2,
            "b1t": b1t, "b2t": b2t, "iota": iota, "ident": ident,
            "idxA": pidxA[k], "idxB": pidxB[k], "idxD": pidxD[k],
            "dstloc": dloc_t[k], "vmask": vm[k],
        })
    return in_maps, out_node


def build(cfg: Cfg):
    import concourse.bacc as bacc
    import concourse.mybir as mybir
    import concourse.tile as tile
    from concourse import library_config
    from contextlib import ExitStack

    f32 = mybir.dt.float32
    bf16 = mybir.dt.bfloat16
    i16 = mybir.dt.int16
    AOP = mybir.AluOpType
    ACTF = mybir.ActivationFunctionType
    X = mybir.AxisListType.X

    NC, NT, NCH, NA, NB = cfg.ncores, cfg.ntile, cfg.nchunk, cfg.na, cfg.nb
    NAC, NBC = NA // 128, NB // 128
    SH, NSLOT, VA, VB = cfg.shslots, cfg.nslot, cfg.va, cfg.vb
    D1, REC, BT = cfg.d1, cfg.rec, cfg.bt
    HEND = 16 + D1 // 2
    NW = D1 + 16
    # RB defined after dram tensors below
    EPS = 1e-16
    CCB = cfg.ccb
    assert NT % CCB == 0
    NCC = NT // CCB           # number of AllGather chunks

    nc = bacc.Bacc('TRN2', target_bir_lowering=False, debug=False, num_devices=NC)

    xT_d = nc.dram_tensor('xT', [128, NSLOT], bf16, kind='ExternalInput')
    Wall1_d = nc.dram_tensor('Wall1', [128, NW], bf16, kind='ExternalInput')
    Wall2_d = nc.dram_tensor('Wall2', [2, 128, NW], bf16, kind='ExternalInput')
    b1t_d = nc.dram_tensor('b1t', [128, D1], f32, kind='ExternalInput')
    b2t_d = nc.dram_tensor('b2t', [128, cfg.out], f32, kind='ExternalInput')
    iota_d = nc.dram_tensor('iota', [128, 128], bf16, kind='ExternalInput')
    ident_d = nc.dram_tensor('ident', [128, 128], bf16, kind='ExternalInput')
    idxA_d = nc.dram_tensor('idxA', [NT, 128, NA // 16], i16, kind='ExternalInput')
    idxB_d = nc.dram_tensor('idxB', [NT, 128, NB // 16], i16, kind='ExternalInput')
    idxD_d = nc.dram_tensor('idxD', [NT, 128, (NA + NB) // 16], i16,
                            kind='ExternalInput')
    dstloc_d = nc.dram_tensor('dstloc', [NT, 128, NCH], bf16, kind='ExternalInput')
    vmask_d = nc.dram_tensor('vmask', [NT, 128, 1], f32, kind='ExternalInput')
    out2_d = nc.dram_tensor('out2', [SH, cfg.out], f32, kind='ExternalOutput')
    RB = 2 * REC          # row length in bf16 units (384 = 768B)
    T1 = nc.dram_tensor('T1', [NSLOT, RB], bf16, kind='Internal')
    HE = 16 + D1          # written row prefix (bf16 cols)
    ccs = [nc.dram_tensor(f'cc{c}', [CCB * 128, RB], bf16, kind='Internal')
           for c in range(NCC)]
    T2 = nc.dram_tensor('T2', [NSLOT, RB], bf16, kind='Internal',
                        addr_space='Shared' if cfg.use_collective else 'Local')

    with tile.TileContext(nc) as tc, ExitStack() as ctx:
        const = ctx.enter_context(tc.tile_pool(name='const', bufs=1))
        nc.gpsimd.load_library(library_config.mlp)

        w1 = const.tile([128, NW], bf16)
        nc.sync.dma_start(w1[:], Wall1_d[:])
        w2 = const.tile([128, 2, NW], bf16)
        nc.sync.dma_start(w2[:], Wall2_d[:].rearrange("k p w -> p k w"))
        b1 = const.tile([128, D1], f32)
        nc.sync.dma_start(b1[:], b1t_d[:])
        b2 = const.tile([128, cfg.out], f32)
        nc.sync.dma_start(b2[:], b2t_d[:])
        iot = const.tile([128, 128], bf16)
        nc.sync.dma_start(iot[:], iota_d[:])
        idn = const.tile([128, 128], bf16)
        nc.sync.dma_start(idn[:], ident_d[:])
        vmt = const.tile([128, NT], f32)
        nc.sync.dma_start(vmt[:], vmask_d[:].rearrange("t p o -> p (t o)"))


        # ---------- phase D1: replicated dense, writes T1 ----------
        with tc.tile_pool(name='dx', bufs=2) as dx, \
             tc.tile_pool(name='dps', bufs=4, space='PSUM') as dps, \
             tc.tile_pool(name='dstg', bufs=3) as dstg:
            ng = NSLOT // 128
            for g0 in range(0, ng, cfg.xbatch):
                gb = min(cfg.xbatch, ng - g0)
                xt = dx.tile([128, gb * 128], bf16, tag='xt')
                nc.sync.dma_start(xt[:], xT_d[:, g0 * 128:(g0 + gb) * 128])
                for w0 in range(0, gb, cfg.wg):
                    wg = min(cfg.wg, gb - w0)
                    stg = dstg.tile([128, wg, HE], bf16, tag=f'stg{wg}')
                    for t in range(wg):
                        ps = dps.tile([128, NW], f32, tag='dps')
                        nc.tensor.matmul(ps[:], xt[:, (w0 + t) * 128:(w0 + t + 1) * 128],
                                         w1[:], start=True, stop=True)
                        if t % 2 == 0:
                            nc.scalar.copy(stg[:, t, :], ps[:])
                        else:
                            nc.vector.tensor_copy(stg[:, t, :], ps[:])
                    g = g0 + w0
                    nc.sync.dma_start(
                        T1[g * 128:(g + wg) * 128, 0:HE].rearrange(
                            "(t p) r -> p t r", p=128),
                        stg[:])

        # ---------- edge phases ----------
        blk = NC * CCB * 128
        ABT = min(NT, (VA // blk) * CCB)   # tiles whose dst rows are in A view

        # idx/dstloc tables are identical for both edge phases: load once
        idxp = ctx.enter_context(tc.tile_pool(name='idx', bufs=1))
        iaL = idxp.tile([128, NT, NA // 16], i16)
        nc.sync.dma_start(iaL[:], idxA_d[:].rearrange("t p w -> p t w"))
        ibL = idxp.tile([128, NT, NB // 16], i16)
        nc.sync.dma_start(ibL[:], idxB_d[:].rearrange("t p w -> p t w"))
        idL = idxp.tile([128, NT, (NA + NB) // 16], i16)
        nc.sync.dma_start(idL[:], idxD_d[:].rearrange("t p w -> p t w"))
        dlL = idxp.tile([128, NT, NCH], bf16)
        nc.sync.dma_start(dlL[:], dstloc_d[:].rearrange("t p w -> p t w"))

        def edge_phase(layer, T, epilogue):
            pname = f'e{layer}'
            with tc.tile_pool(name=pname + 'g', bufs=2) as gp, \
                 tc.tile_pool(name=pname + 'w', bufs=4) as wp, \
                 tc.tile_pool(name=pname + 'o', bufs=4) as op, \
                 tc.tile_pool(name=pname + 'ps', bufs=4, space='PSUM') as pp:
                rowA_src = T[0:VA, :]
                rowB_src = T[VB:NSLOT, :]
                aA_src = T[0:VA, 0:128]
                aB_src = T[VB:NSLOT, 0:128]

                batches = [b for b in range(0, ABT, BT)] + \
                          [b for b in range(ABT, NT, BT)]
                for b0 in batches:
                    bt = min(BT, (ABT if b0 < ABT else NT) - b0)
                    gA = gp.tile([128, bt * NAC, RB], bf16, tag='gA')
                    nc.gpsimd.dma_gather(
                        gA[:], rowA_src,
                        iaL[:, b0:b0 + bt, :].rearrange("p t w -> p (t w)"),
                        bt * NA, bt * NA, RB, single_packet=False)
                    gB = gp.tile([128, bt * NBC, RB], bf16, tag='gB')
                    nc.gpsimd.dma_gather(
                        gB[:], rowB_src,
                        ibL[:, b0:b0 + bt, :].rearrange("p t w -> p (t w)"),
                        bt * NB, bt * NB, RB, single_packet=False)
                    gD = gp.tile([128, bt * NCH, 128], bf16, tag='gD')
                    nc.gpsimd.dma_gather(
                        gD[:], aA_src if b0 < ABT else aB_src,
                        idL[:, b0:b0 + bt, :].rearrange("p t w -> p (t w)"),
                        bt * (NA + NB), bt * (NA + NB), 128, elem_step=RB,
                        single_packet=False)

                    for t in range(bt):
                        tg = b0 + t
                        # one-hots for all chunks of this tile: oh[e, j, slot]
                        ohs = wp.tile([128, NCH, 128], bf16, tag='ohs')
                        nc.vector.tensor_tensor(
                            ohs[:],
                            iot[:].rearrange("p f -> p () f").to_broadcast(
                                [128, NCH, 128]),
                            dlL[:, tg, :].rearrange("p j -> p j ()").to_broadcast(
                                [128, NCH, 128]),
                            op=AOP.is_equal)
                        # z = asrc[src] + adst[dst]; leaky; exp
                        zb = wp.tile([128, NCH * 8], f32, tag='zb')
                        nc.vector.tensor_tensor(
                            zb[:, 0:NAC * 8].rearrange("p (b h) -> p b h", b=NAC),
                            gA[:, t * NAC:(t + 1) * NAC, 0:8],
                            gD[:, t * NCH:t * NCH + NAC, 8:16],
                            op=AOP.add)
                        nc.vector.tensor_tensor(
                            zb[:, NAC * 8:NCH * 8].rearrange("p (b h) -> p b h", b=NBC),
                            gB[:, t * NBC:(t + 1) * NBC, 0:8],
                            gD[:, t * NCH + NAC:(t + 1) * NCH, 8:16],
                            op=AOP.add)
                        zl = wp.tile([128, NCH * 8], f32, tag='zl')
                        nc.vector.scalar_tensor_tensor(
                            zl[:], zb[:], 0.2, zb[:], op0=AOP.mult, op1=AOP.max)
                        p = wp.tile([128, NCH * 8], bf16, tag='p')
                        nc.scalar.activation(p[:], zl[:], ACTF.Exp)
                        # msg = h[src] * p  (h stored (c h)-major: all
                        # operands stride-1 in the last dim -> DVE fast path)
                        msgA = wp.tile([128, NAC, 32, 8], bf16, tag='msgA')
                        nc.vector.tensor_tensor(
                            msgA[:],
                            gA[:, t * NAC:(t + 1) * NAC, 16:16 + D1].rearrange(
                                "p b (c h) -> p b c h", h=8),
                            p[:, 0:NAC * 8].rearrange(
                                "p (b h) -> p b () h", b=NAC).to_broadcast(
                                [128, NAC, 32, 8]),
                            op=AOP.mult)
                        msgB = wp.tile([128, NBC, 32, 8], bf16, tag='msgB')
                        nc.vector.tensor_tensor(
                            msgB[:],
                            gB[:, t * NBC:(t + 1) * NBC, 16:16 + D1].rearrange(
                                "p b (c h) -> p b c h", h=8),
                            p[:, NAC * 8:].rearrange(
                                "p (b h) -> p b () h", b=NBC).to_broadcast(
                                [128, NBC, 32, 8]),
                            op=AOP.mult)
                        # scatter to dst slots
                        paw = pp.tile([128, D1], f32, tag='paw')
                        pdt = pp.tile([128, 8], f32, tag='aux', name='pdt')
                        for j in range(NCH):
                            if j < NAC:
                                rhs = msgA[:, j].rearrange("p c h -> p (c h)")
                            else:
                                rhs = msgB[:, j - NAC].rearrange("p c h -> p (c h)")
                            nc.tensor.matmul(paw[:], ohs[:, j, :], rhs,
                                             start=(j == 0), stop=(j == NCH - 1))
                            nc.tensor.matmul(
                                pdt[:], ohs[:, j, :], p[:, j * 8:(j + 1) * 8],
                                start=(j == 0), stop=(j == NCH - 1))
                        epilogue(tg, paw[:], pdt[:], op, pp)

        # ---------- epilogues ----------
        ccstage = {}

        def epi1(tg, pa, pd, op, pp):
            d1 = op.tile([128, 8], f32, tag='d1')
            nc.vector.tensor_scalar(d1[:], pd, EPS, None, op0=AOP.add)
            r = op.tile([128, 8], f32, tag='r')
            nc.vector.reciprocal(r[:], d1[:])
            o1 = op.tile([128, D1], f32, tag='o1')
            rb = r[:].rearrange("p h -> p () h").to_broadcast([128, 32, 8])
            nc.vector.tensor_tensor(o1[:].rearrange("p (c h) -> p c h", h=8),
                                    pa.rearrange("p (c h) -> p c h", h=8), rb,
                                    op=AOP.mult)
            nc.vector.tensor_tensor(o1[:], o1[:], b1[:], op=AOP.add)
            ex = op.tile([128, D1], f32, tag='ex')
            nc.scalar.activation(ex[:], o1[:], ACTF.Exp)
            nc.vector.tensor_scalar(ex[:], ex[:], 1.0, 1.0, op0=AOP.min,
                                    op1=AOP.subtract)
            et = op.tile([128, D1], bf16, tag='et')
            nc.vector.scalar_tensor_tensor(
                et[:], o1[:], 0.0, ex[:], op0=AOP.max, op1=AOP.add)
            # ---- fused D2: h2 row for this tile -> cc chunk staging ----
            lh = op.tile([128, 2, 128], bf16, tag='lh')
            ptr = pp.tile([128, 2, 128], bf16, tag='aux', name='ptr')
            nc.tensor.transpose(ptr[:, 0], et[:, 0:128], idn[:])
            nc.tensor.transpose(ptr[:, 1], et[:, 128:256], idn[:])
            nc.scalar.copy(lh[:], ptr[:])
            pd2 = pp.tile([128, NW], f32, tag='aux', name='pd2')
            nc.tensor.matmul(pd2[:], lh[:, 0], w2[:, 0], start=True, stop=False)
            nc.tensor.matmul(pd2[:], lh[:, 1], w2[:, 1], start=False, stop=True)
            cci, cto = tg // CCB, tg % CCB
            if cto == 0:
                ccstage[cci] = op.tile([128, CCB, RB], bf16, tag='ccstg', name='ccstg')
                nc.vector.memset(ccstage[cci][:, :, HE:RB], 0.0)
            row = ccstage[cci]
            nc.scalar.activation(row[:, cto, 0:HE], pd2[:],
                                 ACTF.Copy, scale=vmt[:, tg:tg + 1])
            if cto == CCB - 1:
                nc.sync.dma_start(
                    ccs[cci][:].rearrange("(t p) r -> p t r", p=128), row[:])
                del ccstage[cci]
                if cfg.use_collective:
                    blk = NC * CCB * 128
                    nc.gpsimd.collective_compute(
                        "AllGather", mybir.AluOpType.bypass,
                        ins=[ccs[cci][:]],
                        outs=[T2[cci * blk:(cci + 1) * blk, :]],
                        replica_groups=[list(range(NC))],
                    )

        outstage = {}

        def epi2(tg, pa, pd, op, pp):
            d1 = op.tile([128, 8], f32, tag='d1')
            nc.vector.tensor_scalar(d1[:], pd, EPS, None, op0=AOP.add)
            r = op.tile([128, 8], f32, tag='r')
            nc.vector.reciprocal(r[:], d1[:])
            o1 = op.tile([128, D1], f32, tag='o1')
            rb = r[:].rearrange("p h -> p () h").to_broadcast([128, cfg.out, 8])
            nc.vector.tensor_tensor(o1[:].rearrange("p (c h) -> p c h", h=8),
                                    pa.rearrange("p (c h) -> p c h", h=8), rb,
                                    op=AOP.mult)
            m = op.tile([128, cfg.out], f32, tag='m')
            nc.vector.reduce_sum(m[:].rearrange("p c -> p c ()"),
                                 o1[:].rearrange("p (c h) -> p c h", h=8), axis=X)
            cci, cto = tg // CCB, tg % CCB
            if cto == 0:
                outstage[cci] = op.tile([128, CCB, cfg.out], f32, tag='ostg', name='ostg')
            ob = outstage[cci]
            nc.vector.scalar_tensor_tensor(ob[:, cto, :], m[:], 1.0 / cfg.h,
                                           b2[:], op0=AOP.mult, op1=AOP.add)
            if cto == CCB - 1:
                nc.sync.dma_start(
                    out2_d[cci * CCB * 128:(cci + 1) * CCB * 128, :].rearrange(
                        "(t p) c -> p t c", p=128), ob[:])
                del outstage[cci]

        edge_phase(1, T1, epi1)

        if not cfg.use_collective:
            # timing-sim-only stand-in for the AllGather: copy local chunks to
            # the core-0 block of T2 (values wrong cross-core, timing close)
            with tc.tile_pool(name='ccb', bufs=2) as ccbp:
                for cci in range(NCC):
                    bb = ccbp.tile([128, CCB, RB], bf16, tag='bb')
                    nc.sync.dma_start(
                        bb[:], ccs[cci][:].rearrange("(t p) r -> p t r", p=128))
                    nc.sync.dma_start(
                        T2[cci * CCB * 128:(cci + 1) * CCB * 128, :].rearrange(
                            "(t p) r -> p t r", p=128), bb[:])

        edge_phase(2, T2, epi2)

    nc.compile()
    return nc


def np_reference(x, edge_index, W1, as1, ad1, b1, W2, as2, ad2, b2):
    """Pure-numpy GAT reference (matches reference.py semantics)."""
    def conv(x, W, asv, adv, bias, src, dst, N, concat):
        H, C = asv.shape
        h = (x @ W).reshape(-1, H, C)
        a_src = np.einsum("nhc,hc->nh", h, asv)
        a_dst = np.einsum("nhc,hc->nh", h, adv)
        e = a_src[src] + a_dst[dst]
        e = np.where(e > 0, e, 0.2 * e)
        emax = np.full((N, H), -np.inf, dtype=np.float64)
        np.maximum.at(emax, dst, e)
        emax = np.where(np.isfinite(emax), emax, 0.0)
        p = np.exp(e - emax[dst])
        denom = np.zeros((N, H), dtype=np.float64)
        np.add.at(denom, dst, p)
        alpha = p / (denom[dst] + 1e-16)
        msg = h[src] * alpha[:, :, None]
        out = np.zeros((N, H, C), dtype=np.float64)
        np.add.at(out, dst, msg)
        if concat:
            out = out.reshape(N, H * C)
        else:
            out = out.mean(axis=1)
        return out + bias

    x = np.asarray(x, np.float64)
    src, dst = edge_index[0], edge_index[1]
    N = x.shape[0]
    h = conv(x, np.asarray(W1, np.float64), np.asarray(as1, np.float64),
             np.asarray(ad1, np.float64), np.asarray(b1, np.float64),
             src, dst, N, True)
    h = np.where(h > 0, h, np.exp(np.minimum(h, 0)) - 1)
    out = conv(h, np.asarray(W2, np.float64), np.asarray(as2, np.float64),
               np.asarray(ad2, np.float64), np.asarray(b2, np.float64),
               src, dst, N, False)
    return out.astype(np.float32)


_CACHE = {}


def kernel(x, edge_index, W1, att_src1, att_dst1, b1, W2, att_src2,
           att_dst2, b2):
    cfg = Cfg()
    in_maps, out_node = host_prep(cfg, x, edge_index, W1, att_src1,
                                  att_dst1, b1, W2, att_src2, att_dst2, b2)
    if 'nc' not in _CACHE:
        _CACHE['nc'] = build(cfg)
    nc = _CACHE['nc']
    from concourse.bass_utils import run_bass_kernel_spmd
    res = run_bass_kernel_spmd(nc, in_maps, core_ids=list(range(cfg.ncores)))
    full = np.concatenate([res.results[k]['out2'] for k in range(cfg.ncores)],
                          axis=0)
    flat = out_node.reshape(-1)
    out = np.zeros((cfg.n, cfg.out), np.float32)
    real = flat >= 0
    out[flat[real]] = full[real]
    return out



# revision 34
# speedup vs baseline: 1.1770x; 1.1770x over previous
"""GAT 2-layer distributed Bass kernel for TRN2 (8 cores) — v3.

Changes vs v2 baseline:
  - ONE AllGather of the full layer-2 table (rank-major row order) instead
    of 7 chunked AllGathers: small chunks ran at ~40GB/s with a 15us fixed
    cost each, serialized on the collective cores (~1ms total).
  - rank-major global order requires per-core ROTATED local order for T1
    (own shard first), so the dense phase + layer-1 gathers use per-core
    local row ids (input data), layer-2 gathers use global row ids.
  - dst-attention term: own-shard rows are local rows [0, SH) on both
    layers, so the per-edge dst gather shrinks from 256B to 16B elems
    (adst cols only), read from T1 (L1) / the local cc staging (L2).
  - rows shrink 768B -> 576B stride (544B gathered): [asrc 8 | adst 8 |
    h 256] bf16 + 16 pad cols.
  - no reserved dummy slots: pad edges point at row 0; the one-hot
    scatter (dstloc=-1 never matches a lane) masks their contribution.

Table layout per node-slot row (576B = 288 bf16, RB):
  [asrc 8 | adst 8 | h 256 | pad 16] (all bf16)
T1 = layer-1 table, per-core LOCAL order (written by replicated dense)
cc_all = this core's own-shard layer-2 rows (local order, rows 0..SH)
T2 = AllGather(cc_all) -> global rank-major order

Global row id: k*SH + t*128 + p. Core k's local row of node (j,t,p):
((j-k) mod 8)*SH + t*128 + p.
"""
import dataclasses
import numpy as np


@dataclasses.dataclass
class Cfg:
    ncores: int = 8
    ntile: int = 49          # dst tiles per core
    nchunk: int = 9          # 128-edge chunks per tile
    na: int = 512            # A-view edge slots per tile
    nb: int = 640            # B-view edge slots per tile
    n: int = 50000           # real nodes
    e: int = 400000
    fin: int = 128
    h: int = 8
    c: int = 32              # layer-1 head dim (h*c = 256)
    out: int = 32            # layer-2 head dim
    bt: int = 4              # tiles per gather batch
    use_collective: bool = True
    xbatch: int = 56         # dense node-tiles per x-stream DMA
    wg: int = 28             # T1 row-groups per write DMA
    ccb: int = 7             # tiles per cc staging batch

    @property
    def shslots(self):
        return self.ntile * 128

    @property
    def nslot(self):
        return self.ncores * self.shslots

    @property
    def va(self):
        return min(32256, self.nslot)

    @property
    def vb(self):
        return self.nslot - self.va

    @property
    def d1(self):
        return self.h * self.c      # 256

    @property
    def rec(self):
        # 192 f32 per row (768B): gather elem AND row stride must both be
        # multiples of 256B, and [asrc 8|adst 8|h 256] = 544B doesn't fit 512B
        return 16 + self.d1 // 2 + 48


def host_prep(cfg: Cfg, x, edge_index, W1, as1, ad1, b1, W2, as2, ad2, b2):
    N, E = cfg.n, cfg.e
    NC, NT, SH = cfg.ncores, cfg.ntile, cfg.shslots
    src = np.asarray(edge_index[0], dtype=np.int64)
    dst = np.asarray(edge_index[1], dtype=np.int64)
    deg = np.bincount(dst, minlength=N)

    # ---- assign nodes to (core, tile, slot), balancing edge counts ----
    order = np.argsort(-deg, kind="stable")
    core_load = np.zeros(NC, dtype=np.int64)
    core_cnt = np.zeros(NC, dtype=np.int64)
    cap_core = N // NC
    node_core = np.empty(N, dtype=np.int64)
    for nd in order:
        k = np.argmin(np.where(core_cnt < cap_core, core_load, np.iinfo(np.int64).max))
        node_core[nd] = k
        core_load[k] += deg[nd]
        core_cnt[k] += 1

    # node -> (core, tile, lane)
    node_k = np.empty(N, dtype=np.int64)
    node_t = np.empty(N, dtype=np.int64)
    node_p = np.empty(N, dtype=np.int64)
    slot2node = np.full(cfg.nslot, -1, dtype=np.int64)
    for k in range(NC):
        nodes_k = order[node_core[order] == k]
        tcap = np.full(NT, 128, dtype=np.int64)
        tload = np.zeros(NT, dtype=np.int64)
        tcnt = np.zeros(NT, dtype=np.int64)
        tmember = [[] for _ in range(NT)]
        for nd in nodes_k:
            t = np.argmin(np.where(tcnt < tcap, tload, np.iinfo(np.int64).max))
            tmember[t].append(nd)
            tload[t] += deg[nd]
            tcnt[t] += 1
        for t in range(NT):
            for i, nd in enumerate(tmember[t]):
                node_k[nd], node_t[nd], node_p[nd] = k, t, i
                slot2node[k * SH + t * 128 + i] = nd

    grow = node_k * SH + node_t * 128 + node_p     # global row per node

    # ---- per (core, tile) edge lists with per-layer A/B split ----
    ecore = node_k[dst]
    etile = node_t[dst]
    dlane = node_p[dst]

    NA, NB, NCH = cfg.na, cfg.nb, cfg.nchunk
    assert NA + NB == NCH * 128 and NA % 128 == 0 and NB % 128 == 0

    # layer 1 uses local rows (rotate rank blocks by -k), layer 2 global
    idxA = np.zeros((2, NC, NT, NA), dtype=np.int64)
    idxB = np.zeros((2, NC, NT, NB), dtype=np.int64)
    idxD = np.zeros((2, NC, NT, NA + NB), dtype=np.int64)  # local dst row
    dloc = np.full((2, NC, NT, NA + NB), -1.0, dtype=np.float32)

    for k in range(NC):
        srow_local = ((node_k[src] - k) % NC) * SH + node_t[src] * 128 + node_p[src]
        srow_global = grow[src]
        for lay, srow in ((0, srow_local), (1, srow_global)):
            for t in range(NT):
                sel = np.nonzero((ecore == k) & (etile == t))[0]
                ss = srow[sel]
                inA = ss < cfg.va
                inB = ss >= cfg.vb
                flex = inA & inB
                a_list = list(np.nonzero(inA & ~inB)[0])
                b_list = list(np.nonzero(inB & ~inA)[0])
                for i in np.nonzero(flex)[0]:
                    if NA - len(a_list) >= NB - len(b_list):
                        a_list.append(i)
                    else:
                        b_list.append(i)
                if len(a_list) > NA or len(b_list) > NB:
                    raise RuntimeError(
                        f"tile overflow l{lay} core{k} tile{t}: "
                        f"{len(a_list)}/{NA} {len(b_list)}/{NB}")
                for p, i in enumerate(a_list):
                    e_id = sel[i]
                    idxA[lay, k, t, p] = ss[i]
                    idxD[lay, k, t, p] = t * 128 + dlane[e_id]
                    dloc[lay, k, t, p] = dlane[e_id]
                for p, i in enumerate(b_list):
                    e_id = sel[i]
                    idxB[lay, k, t, p] = ss[i] - cfg.vb
                    idxD[lay, k, t, NA + p] = t * 128 + dlane[e_id]
                    dloc[lay, k, t, NA + p] = dlane[e_id]

    def pack16(v, width):
        # v [.., NT, width] int -> [.., NT, 128, width//16] int16 wrapped+replicated
        assert v.shape[-1] == width and width % 16 == 0
        lead = v.shape[:-2]
        r = v.reshape(*lead, -1, width // 16, 16)
        r = np.moveaxis(r, -1, -2)      # [.., NT, 16, width//16]
        reps = (1,) * (len(lead) + 1) + (8, 1)
        r = np.tile(r, reps).astype(np.int16)
        return np.ascontiguousarray(r)

    pidxA = pack16(idxA, NA)
    pidxB = pack16(idxB, NB)
    pidxD = pack16(idxD, NA + NB)

    import ml_dtypes
    # dstloc [2, NC, NT, 128, NCH]: position p = j*128 + lane; pads -> -1
    dloc_t = np.moveaxis(dloc.reshape(2, NC, NT, NCH, 128), -1, -2)
    dloc_t = np.ascontiguousarray(dloc_t.astype(ml_dtypes.bfloat16))

    # validmask [NC, NT, 128, 1] indexed by (core, tile, lane)
    vm = np.zeros((NC, NT, 128, 1), dtype=np.float32)
    for k in range(NC):
        for t in range(NT):
            vm[k, t, :, 0] = slot2node[k * SH + t * 128 + np.arange(128)] >= 0
    vm = np.ascontiguousarray(vm)

    # x permuted to global row order, then per-core rotation of rank blocks
    xp = np.zeros((cfg.nslot, cfg.fin), dtype=np.float32)
    real = slot2node >= 0
    xp[real] = np.asarray(x, dtype=np.float32)[slot2node[real]]
    xp = xp.astype(ml_dtypes.bfloat16)
    xTs = []
    for k in range(NC):
        xk = np.roll(xp.reshape(NC, SH, cfg.fin), -k, axis=0).reshape(
            cfg.nslot, cfg.fin)
        xTs.append(np.ascontiguousarray(xk.T))

    def chmaj(M, hdim, axis):
        # permute an (h, c)-ordered head block to (c, h) order along axis
        M = np.moveaxis(M, axis, 0)
        M = M.reshape(cfg.h, hdim, *M.shape[1:])
        M = np.swapaxes(M, 0, 1).reshape(cfg.h * hdim, *M.shape[2:])
        return np.moveaxis(M, 0, axis)

    def fuse(W, asv, adv, hdim):
        Wa = np.einsum("fhc,hc->fh", W.reshape(W.shape[0], cfg.h, hdim), asv)
        Wd = np.einsum("fhc,hc->fh", W.reshape(W.shape[0], cfg.h, hdim), adv)
        # attn columns first so the dense phase can copy [attn|h] in one op;
        # h-part columns in (c h) order so the DVE msg multiply gets
        # stride-1 last dims on every operand (2x/4x fast path)
        return np.concatenate([Wa, Wd, chmaj(W, hdim, 1)], axis=1)

    Wall1 = fuse(np.asarray(W1, np.float32), np.asarray(as1, np.float32),
                 np.asarray(ad1, np.float32), cfg.c).astype(ml_dtypes.bfloat16)
    Wall2f = fuse(chmaj(np.asarray(W2, np.float32), cfg.c, 0),
                  np.asarray(as2, np.float32),
                  np.asarray(ad2, np.float32), cfg.out)
    Wall2 = np.ascontiguousarray(
        Wall2f.reshape(2, 128, Wall2f.shape[1])).astype(ml_dtypes.bfloat16)

    b1t = np.ascontiguousarray(np.tile(
        chmaj(np.asarray(b1, np.float32), cfg.c, 0),
        (128, 1))).astype(ml_dtypes.bfloat16)
    b2t = np.ascontiguousarray(np.tile(np.asarray(b2, np.float32), (128, 1)))
    iota = np.tile(np.arange(128, dtype=np.float32), (128, 1)).astype(ml_dtypes.bfloat16)
    ident = np.eye(128, dtype=np.float32).astype(ml_dtypes.bfloat16)

    # out2 row (k, t*128+p) -> node id (-1 for padding lanes)
    out_node = np.full((NC, SH), -1, dtype=np.int64)
    for k in range(NC):
        out_node[k] = slot2node[k * SH:(k + 1) * SH]

    in_maps = []
    for k in range(NC):
        in_maps.append({
            "xT": xTs[k], "Wall1": Wall1, "Wall2": Wall2,
            "b1t": b1t, "b2t": b2t, "iota": iota, "ident": ident,
            "idxA1": pidxA[0, k], "idxB1": pidxB[0, k], "idxD1": pidxD[0, k],
            "idxA2": pidxA[1, k], "idxB2": pidxB[1, k], "idxD2": pidxD[1, k],
            "dstloc1": dloc_t[0, k], "dstloc2": dloc_t[1, k],
            "vmask": vm[k],
        })
    return in_maps, out_node


def build(cfg: Cfg):
    import concourse.bacc as bacc
    import concourse.mybir as mybir
    import concourse.tile as tile
    from concourse import library_config
    from contextlib import ExitStack

    f32 = mybir.dt.float32
    bf16 = mybir.dt.bfloat16
    i16 = mybir.dt.int16
    AOP = mybir.AluOpType
    ACTF = mybir.ActivationFunctionType
    X = mybir.AxisListType.X

    NC, NT, NCH, NA, NB = cfg.ncores, cfg.ntile, cfg.nchunk, cfg.na, cfg.nb
    NAC, NBC = NA // 128, NB // 128
    SH, NSLOT, VA, VB = cfg.shslots, cfg.nslot, cfg.va, cfg.vb
    D1, REC, BT = cfg.d1, cfg.rec, cfg.bt
    HE = 16 + D1          # written row prefix (bf16 cols) = 272
    NW = D1 + 16          # fused weight cols = 272
    RB = 2 * REC          # row length in bf16 units (288 = 576B)
    EPS = 1e-16
    CCB = cfg.ccb

    nc = bacc.Bacc('TRN2', target_bir_lowering=False, debug=False, num_devices=NC)

    xT_d = nc.dram_tensor('xT', [128, NSLOT], bf16, kind='ExternalInput')
    Wall1_d = nc.dram_tensor('Wall1', [128, NW], bf16, kind='ExternalInput')
    Wall2_d = nc.dram_tensor('Wall2', [2, 128, NW], bf16, kind='ExternalInput')
    b1t_d = nc.dram_tensor('b1t', [128, D1], bf16, kind='ExternalInput')
    b2t_d = nc.dram_tensor('b2t', [128, cfg.out], f32, kind='ExternalInput')
    iota_d = nc.dram_tensor('iota', [128, 128], bf16, kind='ExternalInput')
    ident_d = nc.dram_tensor('ident', [128, 128], bf16, kind='ExternalInput')
    idx_d = {}
    for lay in (1, 2):
        idx_d[f'idxA{lay}'] = nc.dram_tensor(
            f'idxA{lay}', [NT, 128, NA // 16], i16, kind='ExternalInput')
        idx_d[f'idxB{lay}'] = nc.dram_tensor(
            f'idxB{lay}', [NT, 128, NB // 16], i16, kind='ExternalInput')
        idx_d[f'idxD{lay}'] = nc.dram_tensor(
            f'idxD{lay}', [NT, 128, (NA + NB) // 16], i16, kind='ExternalInput')
        idx_d[f'dstloc{lay}'] = nc.dram_tensor(
            f'dstloc{lay}', [NT, 128, NCH], bf16, kind='ExternalInput')
    vmask_d = nc.dram_tensor('vmask', [NT, 128, 1], f32, kind='ExternalInput')
    out2_d = nc.dram_tensor('out2', [SH, cfg.out], f32, kind='ExternalOutput')
    T1 = nc.dram_tensor('T1', [NSLOT, RB], bf16, kind='Internal')
    # own-shard [asrc|adst|h-head] cols, written early in the dense phase so
    # the layer-1 dst-attention prefetch overlaps the rest of dense
    ownT = nc.dram_tensor('ownT', [SH, 128], bf16, kind='Internal')
    cc_all = nc.dram_tensor('cc_all', [SH, RB], bf16, kind='Internal')
    T2 = nc.dram_tensor('T2', [NSLOT, RB], bf16, kind='Internal',
                        addr_space='Shared' if cfg.use_collective else 'Local')

    with tile.TileContext(nc) as tc, ExitStack() as ctx, \
            nc.allow_low_precision(reason="bf16 epilogue; rel-err gate 2e-2"):
        const = ctx.enter_context(tc.tile_pool(name='const', bufs=1))
        nc.gpsimd.load_library(library_config.mlp)

        w1 = const.tile([128, NW], bf16)
        nc.sync.dma_start(w1[:], Wall1_d[:])
        w2 = const.tile([128, 2, NW], bf16)
        nc.sync.dma_start(w2[:], Wall2_d[:].rearrange("k p w -> p k w"))
        b1 = const.tile([128, D1], bf16)
        nc.sync.dma_start(b1[:], b1t_d[:])
        b2 = const.tile([128, cfg.out], f32)
        nc.sync.dma_start(b2[:], b2t_d[:])
        iot = const.tile([128, 128], bf16)
        nc.sync.dma_start(iot[:], iota_d[:])
        idn = const.tile([128, 128], bf16)
        nc.sync.dma_start(idn[:], ident_d[:])
        vmt = const.tile([128, NT], f32)
        nc.sync.dma_start(vmt[:], vmask_d[:].rearrange("t p o -> p (t o)"))

        # persistent pool for the dst-attention prefetch gathers: allocated
        # BEFORE the dense-phase pools so its SBUF does not alias theirs
        # (aliasing would add an anti-dependency that kills the overlap)
        pfp = ctx.enter_context(tc.tile_pool(name='pf', bufs=2))
        PBT = 4

        # ---------- edge phases ----------
        idxp = ctx.enter_context(tc.tile_pool(name='idx', bufs=1))

        def load_idx(lay):
            iaL = idxp.tile([128, NT, NA // 16], i16, name=f'iaL{lay}')
            nc.sync.dma_start(iaL[:], idx_d[f'idxA{lay}'][:].rearrange("t p w -> p t w"))
            ibL = idxp.tile([128, NT, NB // 16], i16, name=f'ibL{lay}')
            nc.sync.dma_start(ibL[:], idx_d[f'idxB{lay}'][:].rearrange("t p w -> p t w"))
            idL = idxp.tile([128, NT, (NA + NB) // 16], i16, name=f'idL{lay}')
            nc.sync.dma_start(idL[:], idx_d[f'idxD{lay}'][:].rearrange("t p w -> p t w"))
            dlL = idxp.tile([128, NT, NCH], bf16, name=f'dlL{lay}')
            nc.sync.dma_start(dlL[:], idx_d[f'dstloc{lay}'][:].rearrange("t p w -> p t w"))
            return iaL, ibL, idL, dlL

        idx1 = load_idx(1)
        idx2 = load_idx(2)

        # per-edge dst attention, compacted to 8 cols: zda[lay][:, t, j, h]
        zda = {1: idxp.tile([128, NT, NCH, 8], bf16, name='zda1'),
               2: idxp.tile([128, NT, NCH, 8], bf16, name='zda2')}


        # ---------- phase D1: replicated dense, writes T1 (local order) ----
        with tc.tile_pool(name='dx', bufs=2) as dx, \
             tc.tile_pool(name='dps', bufs=4, space='PSUM') as dps, \
             tc.tile_pool(name='dstg', bufs=3) as dstg:
            ng = NSLOT // 128
            for g0 in range(0, ng, cfg.xbatch):
                gb = min(cfg.xbatch, ng - g0)
                xt = dx.tile([128, gb * 128], bf16, tag='xt')
                nc.sync.dma_start(xt[:], xT_d[:, g0 * 128:(g0 + gb) * 128])
                for w0 in range(0, gb, cfg.wg):
                    wg = min(cfg.wg, gb - w0)
                    stg = dstg.tile([128, wg, HE], bf16, tag=f'stg{wg}')
                    for t in range(wg):
                        ps = dps.tile([128, NW], f32, tag='dps')
                        nc.tensor.matmul(ps[:], xt[:, (w0 + t) * 128:(w0 + t + 1) * 128],
                                         w1[:], start=True, stop=True)
                        if t % 2 == 0:
                            nc.scalar.copy(stg[:, t, :], ps[:])
                        else:
                            nc.vector.tensor_copy(stg[:, t, :], ps[:])
                    g = g0 + w0
                    nc.sync.dma_start(
                        T1[g * 128:(g + wg) * 128, 0:HE].rearrange(
                            "(t p) r -> p t r", p=128),
                        stg[:])
                    if g < NT:
                        gc = min(wg, NT - g)
                        nc.sync.dma_start(
                            ownT[g * 128:(g + gc) * 128, :].rearrange(
                                "(t p) r -> p t r", p=128),
                            stg[:, 0:gc, 0:128])

        def prefetch_gd(layer, adst_src, estep, idxt):
            """Gather adst[dst] for every edge slot (256B elems from the
            local own-shard rows; cols 8:16 are the payload) and compact
            into zda. Runs on Pool/DMA only — placed so it overlaps the
            dense phase (layer 1) / the AllGather (layer 2)."""
            idL = idxt[2]
            for b0 in range(0, NT, PBT):
                bt = min(PBT, NT - b0)
                gD = pfp.tile([128, PBT * NCH, 128], bf16, tag='gD')
                nc.gpsimd.dma_gather(
                    gD[:, 0:bt * NCH, :], adst_src,
                    idL[:, b0:b0 + bt, :].rearrange("p t w -> p (t w)"),
                    bt * (NA + NB), bt * (NA + NB), 128, elem_step=estep,
                    single_packet=False)
                nc.scalar.copy(
                    zda[layer][:, b0:b0 + bt, :, :],
                    gD[:, 0:bt * NCH, 8:16].rearrange(
                        "p (t j) h -> p t j h", t=bt))

        def edge_phase(layer, T, idxt, epilogue):
            iaL, ibL, idL, dlL = idxt
            zd = zda[layer]
            pname = f'e{layer}'
            pend = [None]
            with tc.tile_pool(name=pname + 'g', bufs=2) as gp, \
                 tc.tile_pool(name=pname + 'w', bufs=4) as wp, \
                 tc.tile_pool(name=pname + 'o', bufs=4) as op, \
                 tc.tile_pool(name=pname + 'ps', bufs=4, space='PSUM') as pp:
                rowA_src = T[0:VA, :]
                rowB_src = T[VB:NSLOT, :]

                for b0 in range(0, NT, BT):
                    bt = min(BT, NT - b0)
                    gA = gp.tile([128, bt * NAC, RB], bf16, tag='gA')
                    nc.gpsimd.dma_gather(
                        gA[:], rowA_src,
                        iaL[:, b0:b0 + bt, :].rearrange("p t w -> p (t w)"),
                        bt * NA, bt * NA, RB, single_packet=False)
                    gB = gp.tile([128, bt * NBC, RB], bf16, tag='gB')
                    nc.gpsimd.dma_gather(
                        gB[:], rowB_src,
                        ibL[:, b0:b0 + bt, :].rearrange("p t w -> p (t w)"),
                        bt * NB, bt * NB, RB, single_packet=False)

                    for t in range(bt):
                        tg = b0 + t
                        # one-hots for all chunks of this tile: oh[e, j, slot]
                        ohs = wp.tile([128, NCH, 128], bf16, tag='ohs')
                        nc.vector.tensor_tensor(
                            ohs[:],
                            iot[:].rearrange("p f -> p () f").to_broadcast(
                                [128, NCH, 128]),
                            dlL[:, tg, :].rearrange("p j -> p j ()").to_broadcast(
                                [128, NCH, 128]),
                            op=AOP.is_equal)
                        # z = asrc[src] + adst[dst]; leaky; exp
                        zb = wp.tile([128, NCH * 8], bf16, tag='zb')
                        nc.vector.tensor_tensor(
                            zb[:, 0:NAC * 8].rearrange("p (b h) -> p b h", b=NAC),
                            gA[:, t * NAC:(t + 1) * NAC, 0:8],
                            zd[:, tg, 0:NAC, :],
                            op=AOP.add)
                        nc.vector.tensor_tensor(
                            zb[:, NAC * 8:NCH * 8].rearrange("p (b h) -> p b h", b=NBC),
                            gB[:, t * NBC:(t + 1) * NBC, 0:8],
                            zd[:, tg, NAC:NCH, :],
                            op=AOP.add)
                        zl = wp.tile([128, NCH * 8], bf16, tag='zl')
                        nc.vector.scalar_tensor_tensor(
                            zl[:], zb[:], 0.2, zb[:], op0=AOP.mult, op1=AOP.max)
                        p = wp.tile([128, NCH * 8], bf16, tag='p')
                        nc.scalar.activation(p[:], zl[:], ACTF.Exp)
                        # msg = h[src] * p  (h stored (c h)-major: all
                        # operands stride-1 in the last dim -> DVE fast path)
                        msgA = wp.tile([128, NAC, 32, 8], bf16, tag='msgA')
                        nc.vector.tensor_tensor(
                            msgA[:],
                            gA[:, t * NAC:(t + 1) * NAC, 16:16 + D1].rearrange(
                                "p b (c h) -> p b c h", h=8),
                            p[:, 0:NAC * 8].rearrange(
                                "p (b h) -> p b () h", b=NAC).to_broadcast(
                                [128, NAC, 32, 8]),
                            op=AOP.mult)
                        msgB = wp.tile([128, NBC, 32, 8], bf16, tag='msgB')
                        nc.vector.tensor_tensor(
                            msgB[:],
                            gB[:, t * NBC:(t + 1) * NBC, 16:16 + D1].rearrange(
                                "p b (c h) -> p b c h", h=8),
                            p[:, NAC * 8:].rearrange(
                                "p (b h) -> p b () h", b=NBC).to_broadcast(
                                [128, NBC, 32, 8]),
                            op=AOP.mult)
                        # scatter to dst slots
                        paw = pp.tile([128, D1], f32, tag='paw')
                        pdt = pp.tile([128, 8], f32, tag='aux', name='pdt')
                        for j in range(NCH):
                            if j < NAC:
                                rhs = msgA[:, j].rearrange("p c h -> p (c h)")
                            else:
                                rhs = msgB[:, j - NAC].rearrange("p c h -> p (c h)")
                            nc.tensor.matmul(paw[:], ohs[:, j, :], rhs,
                                             start=(j == 0), stop=(j == NCH - 1))
                            nc.tensor.matmul(
                                pdt[:], ohs[:, j, :], p[:, j * 8:(j + 1) * 8],
                                start=(j == 0), stop=(j == NCH - 1))
                        # software-pipelined epilogue: defer by one tile so
                        # the in-order DVE never head-of-line blocks on this
                        # tile's PSUM while the next tile's front work is ready
                        if pend[0] is not None:
                            epilogue(*pend[0])
                        pend[0] = (tg, paw[:], pdt[:], op, pp)
                if pend[0] is not None:
                    epilogue(*pend[0])
                    pend[0] = None

        # ---------- epilogues ----------
        ccstage = {}

        def epi1(tg, pa, pd, op, pp):
            d1 = op.tile([128, 8], f32, tag='d1')
            nc.vector.tensor_scalar(d1[:], pd, EPS, None, op0=AOP.add)
            r = op.tile([128, 8], bf16, tag='r')
            nc.vector.reciprocal(r[:], d1[:])
            # PSUM read on Act (bf16 out) so the DVE chain stays in 2x mode
            pab = op.tile([128, D1], bf16, tag='pab')
            nc.scalar.copy(pab[:], pa)
            o1 = op.tile([128, D1], bf16, tag='o1')
            rb = r[:].rearrange("p h -> p () h").to_broadcast([128, 32, 8])
            nc.vector.tensor_tensor(o1[:].rearrange("p (c h) -> p c h", h=8),
                                    pab[:].rearrange("p (c h) -> p c h", h=8),
                                    rb, op=AOP.mult)
            nc.vector.tensor_tensor(o1[:], o1[:], b1[:], op=AOP.add)
            ex = op.tile([128, D1], bf16, tag='ex')
            nc.scalar.activation(ex[:], o1[:], ACTF.Exp)
            nc.vector.tensor_scalar(ex[:], ex[:], 1.0, 1.0, op0=AOP.min,
                                    op1=AOP.subtract)
            et = op.tile([128, D1], bf16, tag='et')
            nc.vector.scalar_tensor_tensor(
                et[:], o1[:], 0.0, ex[:], op0=AOP.max, op1=AOP.add)
            # ---- fused D2: h2 row for this tile -> cc staging ----
            lh = op.tile([128, 2, 128], bf16, tag='lh')
            ptr = pp.tile([128, 2, 128], bf16, tag='aux', name='ptr')
            nc.tensor.transpose(ptr[:, 0], et[:, 0:128], idn[:])
            nc.tensor.transpose(ptr[:, 1], et[:, 128:256], idn[:])
            nc.scalar.copy(lh[:], ptr[:])
            pd2 = pp.tile([128, NW], f32, tag='aux', name='pd2')
            nc.tensor.matmul(pd2[:], lh[:, 0], w2[:, 0], start=True, stop=False)
            nc.tensor.matmul(pd2[:], lh[:, 1], w2[:, 1], start=False, stop=True)
            cci, cto = tg // CCB, tg % CCB
            if cto == 0:
                ccstage[cci] = op.tile([128, CCB, RB], bf16, tag='ccstg', name='ccstg')
                nc.vector.memset(ccstage[cci][:, :, HE:RB], 0.0)
            row = ccstage[cci]
            nc.scalar.activation(row[:, cto, 0:HE], pd2[:],
                                 ACTF.Copy, scale=vmt[:, tg:tg + 1])
            if cto == CCB - 1:
                nc.sync.dma_start(
                    cc_all[cci * CCB * 128:(cci + 1) * CCB * 128, :].rearrange(
                        "(t p) r -> p t r", p=128), row[:])
                del ccstage[cci]

        outstage = {}

        def epi2(tg, pa, pd, op, pp):
            d1 = op.tile([128, 8], f32, tag='d1')
            nc.vector.tensor_scalar(d1[:], pd, EPS, None, op0=AOP.add)
            r = op.tile([128, 8], bf16, tag='r')
            nc.vector.reciprocal(r[:], d1[:])
            pab = op.tile([128, D1], bf16, tag='pab')
            nc.scalar.copy(pab[:], pa)
            o1 = op.tile([128, D1], bf16, tag='o1')
            rb = r[:].rearrange("p h -> p () h").to_broadcast([128, cfg.out, 8])
            nc.vector.tensor_tensor(o1[:].rearrange("p (c h) -> p c h", h=8),
                                    pab[:].rearrange("p (c h) -> p c h", h=8),
                                    rb, op=AOP.mult)
            m = op.tile([128, cfg.out], f32, tag='m')
            nc.vector.reduce_sum(m[:].rearrange("p c -> p c ()"),
                                 o1[:].rearrange("p (c h) -> p c h", h=8), axis=X)
            cci, cto = tg // CCB, tg % CCB
            if cto == 0:
                outstage[cci] = op.tile([128, CCB, cfg.out], f32, tag='ostg', name='ostg')
            ob = outstage[cci]
            nc.vector.scalar_tensor_tensor(ob[:, cto, :], m[:], 1.0 / cfg.h,
                                           b2[:], op0=AOP.mult, op1=AOP.add)
            if cto == CCB - 1:
                nc.sync.dma_start(
                    out2_d[cci * CCB * 128:(cci + 1) * CCB * 128, :].rearrange(
                        "(t p) c -> p t c", p=128), ob[:])
                del outstage[cci]

        # layer-1 dst-attention prefetch: depends only on the early ownT
        # writes, so it overlaps the rest of the dense phase
        prefetch_gd(1, ownT[0:SH, :], 128, idx1)

        edge_phase(1, T1, idx1, epi1)

        if cfg.use_collective:
            nc.gpsimd.collective_compute(
                "AllGather", mybir.AluOpType.bypass,
                ins=[cc_all[:]],
                outs=[T2[:]],
                replica_groups=[list(range(NC))],
            )
        else:
            # timing-sim-only stand-in for the AllGather: copy local rows to
            # every rank block of T2 (values wrong cross-core, local DMA cost
            # similar to the receive side of the real collective)
            with tc.tile_pool(name='ccb', bufs=2) as ccbp:
                for cci in range(NT // CCB):
                    bb = ccbp.tile([128, CCB, RB], bf16, tag='bb')
                    nc.sync.dma_start(
                        bb[:], cc_all[cci * CCB * 128:(cci + 1) * CCB * 128,
                                      :].rearrange("(t p) r -> p t r", p=128))
                    for k in range(NC):
                        nc.sync.dma_start(
                            T2[k * SH + cci * CCB * 128:
                               k * SH + (cci + 1) * CCB * 128, :].rearrange(
                                "(t p) r -> p t r", p=128), bb[:])

        # layer-2 dst-attention prefetch: issued after the AllGather so it
        # runs inside its window (reads only the local cc staging)
        prefetch_gd(2, cc_all[0:SH, 0:128], RB, idx2)

        edge_phase(2, T2, idx2, epi2)

    nc.compile()
    return nc


_CACHE = {}


def kernel(x, edge_index, W1, att_src1, att_dst1, b1, W2, att_src2,
           att_dst2, b2):
    cfg = Cfg()
    in_maps, out_node = host_prep(cfg, x, edge_index, W1, att_src1,
                                  att_dst1, b1, W2, att_src2, att_dst2, b2)
    if 'nc' not in _CACHE:
        _CACHE['nc'] = build(cfg)
    nc = _CACHE['nc']
    from concourse.bass_utils import run_bass_kernel_spmd
    res = run_bass_kernel_spmd(nc, in_maps, core_ids=list(range(cfg.ncores)))
    full = np.concatenate([res.results[k]['out2'] for k in range(cfg.ncores)],
                          axis=0)
    flat = out_node.reshape(-1)
    out = np.zeros((cfg.n, cfg.out), np.float32)
    real = flat >= 0
    out[flat[real]] = full[real]
    return out


# revision 38
# speedup vs baseline: 1.3699x; 1.1639x over previous
"""GAT 2-layer distributed Bass kernel for TRN2 (8 cores) — v3.

Changes vs v2 baseline:
  - ONE AllGather of the full layer-2 table (rank-major row order) instead
    of 7 chunked AllGathers: small chunks ran at ~40GB/s with a 15us fixed
    cost each, serialized on the collective cores (~1ms total).
  - rank-major global order requires per-core ROTATED local order for T1
    (own shard first), so the dense phase + layer-1 gathers use per-core
    local row ids (input data), layer-2 gathers use global row ids.
  - dst-attention term: own-shard rows are local rows [0, SH) on both
    layers, so the per-edge dst gather shrinks from 256B to 16B elems
    (adst cols only), read from T1 (L1) / the local cc staging (L2).
  - rows shrink 768B -> 576B stride (544B gathered): [asrc 8 | adst 8 |
    h 256] bf16 + 16 pad cols.
  - no reserved dummy slots: pad edges point at row 0; the one-hot
    scatter (dstloc=-1 never matches a lane) masks their contribution.

Table layout per node-slot row (576B = 288 bf16, RB):
  [asrc 8 | adst 8 | h 256 | pad 16] (all bf16)
T1 = layer-1 table, per-core LOCAL order (written by replicated dense)
cc_all = this core's own-shard layer-2 rows (local order, rows 0..SH)
T2 = AllGather(cc_all) -> global rank-major order

Global row id: k*SH + t*128 + p. Core k's local row of node (j,t,p):
((j-k) mod 8)*SH + t*128 + p.
"""
import dataclasses
import numpy as np


@dataclasses.dataclass
class Cfg:
    ncores: int = 8
    ntile: int = 49          # dst tiles per core
    nchunk: int = 9          # 128-edge chunks per tile
    na: int = 512            # A-view edge slots per tile
    nb: int = 640            # B-view edge slots per tile
    n: int = 50000           # real nodes
    e: int = 400000
    fin: int = 128
    h: int = 8
    c: int = 32              # layer-1 head dim (h*c = 256)
    out: int = 32            # layer-2 head dim
    bt: int = 4              # tiles per gather batch
    use_collective: bool = True
    xbatch: int = 56         # dense node-tiles per x-stream DMA
    wg: int = 28             # T1 row-groups per write DMA
    ccb: int = 7             # tiles per cc staging batch

    @property
    def shslots(self):
        return self.ntile * 128

    @property
    def nslot(self):
        return self.ncores * self.shslots

    @property
    def va(self):
        return min(32256, self.nslot)

    @property
    def vb(self):
        return self.nslot - self.va

    @property
    def d1(self):
        return self.h * self.c      # 256

    @property
    def rec(self):
        # 192 f32 per row (768B): gather elem AND row stride must both be
        # multiples of 256B, and [asrc 8|adst 8|h 256] = 544B doesn't fit 512B
        return 16 + self.d1 // 2 + 48


def host_prep(cfg: Cfg, x, edge_index, W1, as1, ad1, b1, W2, as2, ad2, b2):
    N, E = cfg.n, cfg.e
    NC, NT, SH = cfg.ncores, cfg.ntile, cfg.shslots
    src = np.asarray(edge_index[0], dtype=np.int64)
    dst = np.asarray(edge_index[1], dtype=np.int64)
    deg = np.bincount(dst, minlength=N)

    # ---- assign nodes to (core, tile, slot), balancing edge counts ----
    order = np.argsort(-deg, kind="stable")
    core_load = np.zeros(NC, dtype=np.int64)
    core_cnt = np.zeros(NC, dtype=np.int64)
    cap_core = N // NC
    node_core = np.empty(N, dtype=np.int64)
    for nd in order:
        k = np.argmin(np.where(core_cnt < cap_core, core_load, np.iinfo(np.int64).max))
        node_core[nd] = k
        core_load[k] += deg[nd]
        core_cnt[k] += 1

    # node -> (core, tile, lane)
    node_k = np.empty(N, dtype=np.int64)
    node_t = np.empty(N, dtype=np.int64)
    node_p = np.empty(N, dtype=np.int64)
    slot2node = np.full(cfg.nslot, -1, dtype=np.int64)
    for k in range(NC):
        nodes_k = order[node_core[order] == k]
        tcap = np.full(NT, 128, dtype=np.int64)
        tload = np.zeros(NT, dtype=np.int64)
        tcnt = np.zeros(NT, dtype=np.int64)
        tmember = [[] for _ in range(NT)]
        for nd in nodes_k:
            t = np.argmin(np.where(tcnt < tcap, tload, np.iinfo(np.int64).max))
            tmember[t].append(nd)
            tload[t] += deg[nd]
            tcnt[t] += 1
        for t in range(NT):
            for i, nd in enumerate(tmember[t]):
                node_k[nd], node_t[nd], node_p[nd] = k, t, i
                slot2node[k * SH + t * 128 + i] = nd

    grow = node_k * SH + node_t * 128 + node_p     # global row per node

    # ---- per (core, tile) edge lists with per-layer A/B split ----
    ecore = node_k[dst]
    etile = node_t[dst]
    dlane = node_p[dst]

    NA, NB, NCH = cfg.na, cfg.nb, cfg.nchunk
    assert NA + NB == NCH * 128 and NA % 128 == 0 and NB % 128 == 0

    # layer 1 uses local rows (rotate rank blocks by -k), layer 2 global
    idxA = np.zeros((2, NC, NT, NA), dtype=np.int64)
    idxB = np.zeros((2, NC, NT, NB), dtype=np.int64)
    idxD = np.zeros((2, NC, NT, NA + NB), dtype=np.int64)  # local dst row
    dloc = np.full((2, NC, NT, NA + NB), -1.0, dtype=np.float32)

    for k in range(NC):
        srow_local = ((node_k[src] - k) % NC) * SH + node_t[src] * 128 + node_p[src]
        srow_global = grow[src]
        for lay, srow in ((0, srow_local), (1, srow_global)):
            for t in range(NT):
                sel = np.nonzero((ecore == k) & (etile == t))[0]
                ss = srow[sel]
                inA = ss < cfg.va
                inB = ss >= cfg.vb
                flex = inA & inB
                a_list = list(np.nonzero(inA & ~inB)[0])
                b_list = list(np.nonzero(inB & ~inA)[0])
                for i in np.nonzero(flex)[0]:
                    if NA - len(a_list) >= NB - len(b_list):
                        a_list.append(i)
                    else:
                        b_list.append(i)
                if len(a_list) > NA or len(b_list) > NB:
                    raise RuntimeError(
                        f"tile overflow l{lay} core{k} tile{t}: "
                        f"{len(a_list)}/{NA} {len(b_list)}/{NB}")
                for p, i in enumerate(a_list):
                    e_id = sel[i]
                    idxA[lay, k, t, p] = ss[i]
                    idxD[lay, k, t, p] = t * 128 + dlane[e_id]
                    dloc[lay, k, t, p] = dlane[e_id]
                for p, i in enumerate(b_list):
                    e_id = sel[i]
                    idxB[lay, k, t, p] = ss[i] - cfg.vb
                    idxD[lay, k, t, NA + p] = t * 128 + dlane[e_id]
                    dloc[lay, k, t, NA + p] = dlane[e_id]

    def pack16(v, width):
        # v [.., NT, width] int -> [.., NT, 128, width//16] int16 wrapped+replicated
        assert v.shape[-1] == width and width % 16 == 0
        lead = v.shape[:-2]
        r = v.reshape(*lead, -1, width // 16, 16)
        r = np.moveaxis(r, -1, -2)      # [.., NT, 16, width//16]
        reps = (1,) * (len(lead) + 1) + (8, 1)
        r = np.tile(r, reps).astype(np.int16)
        return np.ascontiguousarray(r)

    pidxA = pack16(idxA, NA)
    pidxB = pack16(idxB, NB)
    pidxD = pack16(idxD, NA + NB)

    import ml_dtypes
    # dstloc [2, NC, NT, 128, NCH, 2]: position p = j*128 + lane; pads -> -1.
    # Each value is duplicated into a stride-1 pair so the one-hot is_equal
    # can present a stride-1 last dim on every operand (DVE 2x mode).
    dloc_t = np.moveaxis(dloc.reshape(2, NC, NT, NCH, 128), -1, -2)
    dloc_t = np.repeat(dloc_t[..., None], 2, axis=-1)
    dloc_t = np.ascontiguousarray(dloc_t.astype(ml_dtypes.bfloat16))

    # validmask [NC, NT, 128, 1] indexed by (core, tile, lane)
    vm = np.zeros((NC, NT, 128, 1), dtype=np.float32)
    for k in range(NC):
        for t in range(NT):
            vm[k, t, :, 0] = slot2node[k * SH + t * 128 + np.arange(128)] >= 0
    vm = np.ascontiguousarray(vm)

    # x permuted to global row order, then per-core rotation of rank blocks
    xp = np.zeros((cfg.nslot, cfg.fin), dtype=np.float32)
    real = slot2node >= 0
    xp[real] = np.asarray(x, dtype=np.float32)[slot2node[real]]
    xp = xp.astype(ml_dtypes.bfloat16)
    xTs = []
    for k in range(NC):
        xk = np.roll(xp.reshape(NC, SH, cfg.fin), -k, axis=0).reshape(
            cfg.nslot, cfg.fin)
        xTs.append(np.ascontiguousarray(xk.T))

    def chmaj(M, hdim, axis):
        # permute an (h, c)-ordered head block to (c, h) order along axis
        M = np.moveaxis(M, axis, 0)
        M = M.reshape(cfg.h, hdim, *M.shape[1:])
        M = np.swapaxes(M, 0, 1).reshape(cfg.h * hdim, *M.shape[2:])
        return np.moveaxis(M, 0, axis)

    def fuse(W, asv, adv, hdim):
        Wa = np.einsum("fhc,hc->fh", W.reshape(W.shape[0], cfg.h, hdim), asv)
        Wd = np.einsum("fhc,hc->fh", W.reshape(W.shape[0], cfg.h, hdim), adv)
        # attn columns first so the dense phase can copy [attn|h] in one op;
        # h-part columns in (c h) order so the DVE msg multiply gets
        # stride-1 last dims on every operand (2x/4x fast path)
        return np.concatenate([Wa, Wd, chmaj(W, hdim, 1)], axis=1)

    Wall1 = fuse(np.asarray(W1, np.float32), np.asarray(as1, np.float32),
                 np.asarray(ad1, np.float32), cfg.c).astype(ml_dtypes.bfloat16)
    Wall2f = fuse(chmaj(np.asarray(W2, np.float32), cfg.c, 0),
                  np.asarray(as2, np.float32),
                  np.asarray(ad2, np.float32), cfg.out)
    Wall2 = np.ascontiguousarray(
        Wall2f.reshape(2, 128, Wall2f.shape[1])).astype(ml_dtypes.bfloat16)

    b1t = np.ascontiguousarray(np.tile(
        chmaj(np.asarray(b1, np.float32), cfg.c, 0),
        (128, 1))).astype(ml_dtypes.bfloat16)
    b2t = np.ascontiguousarray(np.tile(np.asarray(b2, np.float32), (128, 1)))
    iota = np.tile(np.arange(128, dtype=np.float32), (128, 1)).astype(ml_dtypes.bfloat16)
    ident = np.eye(128, dtype=np.float32).astype(ml_dtypes.bfloat16)

    # out2 row (k, t*128+p) -> node id (-1 for padding lanes)
    out_node = np.full((NC, SH), -1, dtype=np.int64)
    for k in range(NC):
        out_node[k] = slot2node[k * SH:(k + 1) * SH]

    in_maps = []
    for k in range(NC):
        in_maps.append({
            "xT": xTs[k], "Wall1": Wall1, "Wall2": Wall2,
            "b1t": b1t, "b2t": b2t, "iota": iota, "ident": ident,
            "idxA1": pidxA[0, k], "idxB1": pidxB[0, k], "idxD1": pidxD[0, k],
            "idxA2": pidxA[1, k], "idxB2": pidxB[1, k], "idxD2": pidxD[1, k],
            "dstloc1": dloc_t[0, k], "dstloc2": dloc_t[1, k],
            "vmask": vm[k],
        })
    return in_maps, out_node


def build(cfg: Cfg):
    import concourse.bacc as bacc
    import concourse.mybir as mybir
    import concourse.tile as tile
    from concourse import library_config
    from contextlib import ExitStack

    f32 = mybir.dt.float32
    bf16 = mybir.dt.bfloat16
    i16 = mybir.dt.int16
    AOP = mybir.AluOpType
    ACTF = mybir.ActivationFunctionType
    X = mybir.AxisListType.X

    NC, NT, NCH, NA, NB = cfg.ncores, cfg.ntile, cfg.nchunk, cfg.na, cfg.nb
    NAC, NBC = NA // 128, NB // 128
    SH, NSLOT, VA, VB = cfg.shslots, cfg.nslot, cfg.va, cfg.vb
    D1, REC, BT = cfg.d1, cfg.rec, cfg.bt
    HE = 16 + D1          # written row prefix (bf16 cols) = 272
    NW = D1 + 16          # fused weight cols = 272
    RB = 2 * REC          # row length in bf16 units (288 = 576B)
    EPS = 1e-16
    CCB = cfg.ccb

    nc = bacc.Bacc('TRN2', target_bir_lowering=False, debug=False, num_devices=NC)

    xT_d = nc.dram_tensor('xT', [128, NSLOT], bf16, kind='ExternalInput')
    Wall1_d = nc.dram_tensor('Wall1', [128, NW], bf16, kind='ExternalInput')
    Wall2_d = nc.dram_tensor('Wall2', [2, 128, NW], bf16, kind='ExternalInput')
    b1t_d = nc.dram_tensor('b1t', [128, D1], bf16, kind='ExternalInput')
    b2t_d = nc.dram_tensor('b2t', [128, cfg.out], f32, kind='ExternalInput')
    iota_d = nc.dram_tensor('iota', [128, 128], bf16, kind='ExternalInput')
    ident_d = nc.dram_tensor('ident', [128, 128], bf16, kind='ExternalInput')
    idx_d = {}
    for lay in (1, 2):
        idx_d[f'idxA{lay}'] = nc.dram_tensor(
            f'idxA{lay}', [NT, 128, NA // 16], i16, kind='ExternalInput')
        idx_d[f'idxB{lay}'] = nc.dram_tensor(
            f'idxB{lay}', [NT, 128, NB // 16], i16, kind='ExternalInput')
        idx_d[f'idxD{lay}'] = nc.dram_tensor(
            f'idxD{lay}', [NT, 128, (NA + NB) // 16], i16, kind='ExternalInput')
        idx_d[f'dstloc{lay}'] = nc.dram_tensor(
            f'dstloc{lay}', [NT, 128, NCH, 2], bf16, kind='ExternalInput')
    vmask_d = nc.dram_tensor('vmask', [NT, 128, 1], f32, kind='ExternalInput')
    out2_d = nc.dram_tensor('out2', [SH, cfg.out], f32, kind='ExternalOutput')
    T1 = nc.dram_tensor('T1', [NSLOT, RB], bf16, kind='Internal')
    # own-shard [asrc|adst|h-head] cols, written early in the dense phase so
    # the layer-1 dst-attention prefetch overlaps the rest of dense
    ownT = nc.dram_tensor('ownT', [SH, 128], bf16, kind='Internal')
    cc_all = nc.dram_tensor('cc_all', [SH, RB], bf16, kind='Internal')
    T2 = nc.dram_tensor('T2', [NSLOT, RB], bf16, kind='Internal',
                        addr_space='Shared' if cfg.use_collective else 'Local')

    with tile.TileContext(nc) as tc, ExitStack() as ctx, \
            nc.allow_low_precision(reason="bf16 epilogue; rel-err gate 2e-2"):
        const = ctx.enter_context(tc.tile_pool(name='const', bufs=1))
        nc.gpsimd.load_library(library_config.mlp)

        w1 = const.tile([128, NW], bf16)
        nc.sync.dma_start(w1[:], Wall1_d[:])
        w2 = const.tile([128, 2, NW], bf16)
        nc.sync.dma_start(w2[:], Wall2_d[:].rearrange("k p w -> p k w"))
        b1 = const.tile([128, D1], bf16)
        nc.sync.dma_start(b1[:], b1t_d[:])
        b2 = const.tile([128, cfg.out], f32)
        nc.sync.dma_start(b2[:], b2t_d[:])
        iot = const.tile([128, 128], bf16)
        nc.sync.dma_start(iot[:], iota_d[:])
        idn = const.tile([128, 128], bf16)
        nc.sync.dma_start(idn[:], ident_d[:])
        vmt = const.tile([128, NT], f32)
        nc.sync.dma_start(vmt[:], vmask_d[:].rearrange("t p o -> p (t o)"))

        # persistent pool for the dst-attention prefetch gathers: allocated
        # BEFORE the dense-phase pools so its SBUF does not alias theirs
        # (aliasing would add an anti-dependency that kills the overlap)
        pfp = ctx.enter_context(tc.tile_pool(name='pf', bufs=2))
        PBT = 4

        # ---------- edge phases ----------
        idxp = ctx.enter_context(tc.tile_pool(name='idx', bufs=1))

        def load_idx(lay):
            iaL = idxp.tile([128, NT, NA // 16], i16, name=f'iaL{lay}')
            nc.sync.dma_start(iaL[:], idx_d[f'idxA{lay}'][:].rearrange("t p w -> p t w"))
            ibL = idxp.tile([128, NT, NB // 16], i16, name=f'ibL{lay}')
            nc.sync.dma_start(ibL[:], idx_d[f'idxB{lay}'][:].rearrange("t p w -> p t w"))
            idL = idxp.tile([128, NT, (NA + NB) // 16], i16, name=f'idL{lay}')
            nc.sync.dma_start(idL[:], idx_d[f'idxD{lay}'][:].rearrange("t p w -> p t w"))
            dlL = idxp.tile([128, NT, NCH, 2], bf16, name=f'dlL{lay}')
            nc.sync.dma_start(dlL[:], idx_d[f'dstloc{lay}'][:].rearrange(
                "t p w two -> p t w two"))
            return iaL, ibL, idL, dlL

        idx1 = load_idx(1)
        idx2 = load_idx(2)

        # per-edge dst attention, compacted to 8 cols: zda[lay][:, t, j, h]
        zda = {1: idxp.tile([128, NT, NCH, 8], bf16, name='zda1'),
               2: idxp.tile([128, NT, NCH, 8], bf16, name='zda2')}


        # ---------- phase D1: replicated dense, writes T1 (local order) ----
        with tc.tile_pool(name='dx', bufs=2) as dx, \
             tc.tile_pool(name='dps', bufs=4, space='PSUM') as dps, \
             tc.tile_pool(name='dstg', bufs=3) as dstg:
            ng = NSLOT // 128
            for g0 in range(0, ng, cfg.xbatch):
                gb = min(cfg.xbatch, ng - g0)
                xt = dx.tile([128, gb * 128], bf16, tag='xt')
                nc.sync.dma_start(xt[:], xT_d[:, g0 * 128:(g0 + gb) * 128])
                for w0 in range(0, gb, cfg.wg):
                    wg = min(cfg.wg, gb - w0)
                    stg = dstg.tile([128, wg, HE], bf16, tag=f'stg{wg}')
                    for t in range(wg):
                        ps = dps.tile([128, NW], f32, tag='dps')
                        nc.tensor.matmul(ps[:], xt[:, (w0 + t) * 128:(w0 + t + 1) * 128],
                                         w1[:], start=True, stop=True)
                        if t % 2 == 0:
                            nc.scalar.copy(stg[:, t, :], ps[:])
                        else:
                            nc.vector.tensor_copy(stg[:, t, :], ps[:])
                    g = g0 + w0
                    nc.sync.dma_start(
                        T1[g * 128:(g + wg) * 128, 0:HE].rearrange(
                            "(t p) r -> p t r", p=128),
                        stg[:])
                    if g < NT:
                        gc = min(wg, NT - g)
                        nc.sync.dma_start(
                            ownT[g * 128:(g + gc) * 128, :].rearrange(
                                "(t p) r -> p t r", p=128),
                            stg[:, 0:gc, 0:128])

        def prefetch_gd(layer, adst_src, estep, idxt):
            """Gather adst[dst] for every edge slot (256B elems from the
            local own-shard rows; cols 8:16 are the payload) and compact
            into zda. Runs on Pool/DMA only — placed so it overlaps the
            dense phase (layer 1) / the AllGather (layer 2)."""
            idL = idxt[2]
            for b0 in range(0, NT, PBT):
                bt = min(PBT, NT - b0)
                gD = pfp.tile([128, PBT * NCH, 128], bf16, tag='gD')
                nc.gpsimd.dma_gather(
                    gD[:, 0:bt * NCH, :], adst_src,
                    idL[:, b0:b0 + bt, :].rearrange("p t w -> p (t w)"),
                    bt * (NA + NB), bt * (NA + NB), 128, elem_step=estep,
                    single_packet=False)
                nc.scalar.copy(
                    zda[layer][:, b0:b0 + bt, :, :],
                    gD[:, 0:bt * NCH, 8:16].rearrange(
                        "p (t j) h -> p t j h", t=bt))

        def edge_phase(layer, T, idxt, epilogue):
            iaL, ibL, idL, dlL = idxt
            zd = zda[layer]
            pname = f'e{layer}'
            pend = [None]
            with tc.tile_pool(name=pname + 'g', bufs=2) as gp, \
                 tc.tile_pool(name=pname + 'w', bufs=4) as wp, \
                 tc.tile_pool(name=pname + 'o', bufs=4) as op, \
                 tc.tile_pool(name=pname + 'ps', bufs=4, space='PSUM') as pp:
                rowA_src = T[0:VA, :]
                rowB_src = T[VB:NSLOT, :]

                for b0 in range(0, NT, BT):
                    bt = min(BT, NT - b0)
                    gA = gp.tile([128, bt * NAC, RB], bf16, tag='gA')
                    nc.gpsimd.dma_gather(
                        gA[:], rowA_src,
                        iaL[:, b0:b0 + bt, :].rearrange("p t w -> p (t w)"),
                        bt * NA, bt * NA, RB, single_packet=False)
                    gB = gp.tile([128, bt * NBC, RB], bf16, tag='gB')
                    nc.gpsimd.dma_gather(
                        gB[:], rowB_src,
                        ibL[:, b0:b0 + bt, :].rearrange("p t w -> p (t w)"),
                        bt * NB, bt * NB, RB, single_packet=False)

                    for t in range(bt):
                        tg = b0 + t
                        # one-hots for all chunks of this tile: oh[e, j, slot]
                        ohs = wp.tile([128, NCH, 128], bf16, tag='ohs')
                        # stride-1 pair view on every operand -> DVE 2x mode
                        nc.vector.tensor_tensor(
                            ohs[:].rearrange("p j (s two) -> p j s two", two=2),
                            iot[:].rearrange("p (s two) -> p () s two", two=2)
                            .to_broadcast([128, NCH, 64, 2]),
                            dlL[:, tg, :, :].rearrange("p j two -> p j () two")
                            .to_broadcast([128, NCH, 64, 2]),
                            op=AOP.is_equal)
                        # z = asrc[src] + adst[dst]; leaky; exp
                        zb = wp.tile([128, NCH * 8], bf16, tag='zb')
                        nc.vector.tensor_tensor(
                            zb[:, 0:NAC * 8].rearrange("p (b h) -> p b h", b=NAC),
                            gA[:, t * NAC:(t + 1) * NAC, 0:8],
                            zd[:, tg, 0:NAC, :],
                            op=AOP.add)
                        nc.vector.tensor_tensor(
                            zb[:, NAC * 8:NCH * 8].rearrange("p (b h) -> p b h", b=NBC),
                            gB[:, t * NBC:(t + 1) * NBC, 0:8],
                            zd[:, tg, NAC:NCH, :],
                            op=AOP.add)
                        zl = wp.tile([128, NCH * 8], bf16, tag='zl')
                        nc.vector.scalar_tensor_tensor(
                            zl[:], zb[:], 0.2, zb[:], op0=AOP.mult, op1=AOP.max)
                        p = wp.tile([128, NCH * 8], bf16, tag='p')
                        nc.scalar.activation(p[:], zl[:], ACTF.Exp)
                        # msg = h[src] * p  (h stored (c h)-major: all
                        # operands stride-1 in the last dim -> DVE fast path)
                        msgA = wp.tile([128, NAC, 32, 8], bf16, tag='msgA')
                        nc.vector.tensor_tensor(
                            msgA[:],
                            gA[:, t * NAC:(t + 1) * NAC, 16:16 + D1].rearrange(
                                "p b (c h) -> p b c h", h=8),
                            p[:, 0:NAC * 8].rearrange(
                                "p (b h) -> p b () h", b=NAC).to_broadcast(
                                [128, NAC, 32, 8]),
                            op=AOP.mult)
                        msgB = wp.tile([128, NBC, 32, 8], bf16, tag='msgB')
                        nc.vector.tensor_tensor(
                            msgB[:],
                            gB[:, t * NBC:(t + 1) * NBC, 16:16 + D1].rearrange(
                                "p b (c h) -> p b c h", h=8),
                            p[:, NAC * 8:].rearrange(
                                "p (b h) -> p b () h", b=NBC).to_broadcast(
                                [128, NBC, 32, 8]),
                            op=AOP.mult)
                        # scatter to dst slots
                        paw = pp.tile([128, D1], f32, tag='paw')
                        pdt = pp.tile([128, 8], f32, tag='aux', name='pdt')
                        for j in range(NCH):
                            if j < NAC:
                                rhs = msgA[:, j].rearrange("p c h -> p (c h)")
                            else:
                                rhs = msgB[:, j - NAC].rearrange("p c h -> p (c h)")
                            nc.tensor.matmul(paw[:], ohs[:, j, :], rhs,
                                             start=(j == 0), stop=(j == NCH - 1))
                            nc.tensor.matmul(
                                pdt[:], ohs[:, j, :], p[:, j * 8:(j + 1) * 8],
                                start=(j == 0), stop=(j == NCH - 1))
                        # software-pipelined epilogue: defer by one tile so
                        # the in-order DVE never head-of-line blocks on this
                        # tile's PSUM while the next tile's front work is ready
                        if pend[0] is not None:
                            epilogue(*pend[0])
                        pend[0] = (tg, paw[:], pdt[:], op, pp)
                if pend[0] is not None:
                    epilogue(*pend[0])
                    pend[0] = None

        # ---------- epilogues ----------
        ccstage = {}

        def epi1(tg, pa, pd, op, pp):
            d1 = op.tile([128, 8], f32, tag='d1')
            nc.vector.tensor_scalar(d1[:], pd, EPS, None, op0=AOP.add)
            r = op.tile([128, 8], bf16, tag='r')
            nc.vector.reciprocal(r[:], d1[:])
            # PSUM read on Act (bf16 out) so the DVE chain stays in 2x mode
            pab = op.tile([128, D1], bf16, tag='pab')
            nc.scalar.copy(pab[:], pa)
            o1 = op.tile([128, D1], bf16, tag='o1')
            rb = r[:].rearrange("p h -> p () h").to_broadcast([128, 32, 8])
            nc.vector.tensor_tensor(o1[:].rearrange("p (c h) -> p c h", h=8),
                                    pab[:].rearrange("p (c h) -> p c h", h=8),
                                    rb, op=AOP.mult)
            nc.vector.tensor_tensor(o1[:], o1[:], b1[:], op=AOP.add)
            ex = op.tile([128, D1], bf16, tag='ex')
            nc.scalar.activation(ex[:], o1[:], ACTF.Exp)
            nc.vector.tensor_scalar(ex[:], ex[:], 1.0, 1.0, op0=AOP.min,
                                    op1=AOP.subtract)
            et = op.tile([128, D1], bf16, tag='et')
            nc.vector.scalar_tensor_tensor(
                et[:], o1[:], 0.0, ex[:], op0=AOP.max, op1=AOP.add)
            # ---- fused D2: h2 row for this tile -> cc staging ----
            lh = op.tile([128, 2, 128], bf16, tag='lh')
            ptr = pp.tile([128, 2, 128], bf16, tag='aux', name='ptr')
            nc.tensor.transpose(ptr[:, 0], et[:, 0:128], idn[:])
            nc.tensor.transpose(ptr[:, 1], et[:, 128:256], idn[:])
            nc.scalar.copy(lh[:], ptr[:])
            pd2 = pp.tile([128, NW], f32, tag='aux', name='pd2')
            nc.tensor.matmul(pd2[:], lh[:, 0], w2[:, 0], start=True, stop=False)
            nc.tensor.matmul(pd2[:], lh[:, 1], w2[:, 1], start=False, stop=True)
            cci, cto = tg // CCB, tg % CCB
            if cto == 0:
                ccstage[cci] = op.tile([128, CCB, RB], bf16, tag='ccstg', name='ccstg')
                nc.vector.memset(ccstage[cci][:, :, HE:RB], 0.0)
            row = ccstage[cci]
            nc.scalar.activation(row[:, cto, 0:HE], pd2[:],
                                 ACTF.Copy, scale=vmt[:, tg:tg + 1])
            if cto == CCB - 1:
                nc.sync.dma_start(
                    cc_all[cci * CCB * 128:(cci + 1) * CCB * 128, :].rearrange(
                        "(t p) r -> p t r", p=128), row[:])
                del ccstage[cci]

        outstage = {}

        def epi2(tg, pa, pd, op, pp):
            d1 = op.tile([128, 8], f32, tag='d1')
            nc.vector.tensor_scalar(d1[:], pd, EPS, None, op0=AOP.add)
            r = op.tile([128, 8], bf16, tag='r')
            nc.vector.reciprocal(r[:], d1[:])
            pab = op.tile([128, D1], bf16, tag='pab')
            nc.scalar.copy(pab[:], pa)
            o1 = op.tile([128, D1], bf16, tag='o1')
            rb = r[:].rearrange("p h -> p () h").to_broadcast([128, cfg.out, 8])
            nc.vector.tensor_tensor(o1[:].rearrange("p (c h) -> p c h", h=8),
                                    pab[:].rearrange("p (c h) -> p c h", h=8),
                                    rb, op=AOP.mult)
            m = op.tile([128, cfg.out], f32, tag='m')
            nc.vector.reduce_sum(m[:].rearrange("p c -> p c ()"),
                                 o1[:].rearrange("p (c h) -> p c h", h=8), axis=X)
            cci, cto = tg // CCB, tg % CCB
            if cto == 0:
                outstage[cci] = op.tile([128, CCB, cfg.out], f32, tag='ostg', name='ostg')
            ob = outstage[cci]
            nc.vector.scalar_tensor_tensor(ob[:, cto, :], m[:], 1.0 / cfg.h,
                                           b2[:], op0=AOP.mult, op1=AOP.add)
            if cto == CCB - 1:
                nc.sync.dma_start(
                    out2_d[cci * CCB * 128:(cci + 1) * CCB * 128, :].rearrange(
                        "(t p) c -> p t c", p=128), ob[:])
                del outstage[cci]

        # layer-1 dst-attention prefetch: depends only on the early ownT
        # writes, so it overlaps the rest of the dense phase
        prefetch_gd(1, ownT[0:SH, :], 128, idx1)

        edge_phase(1, T1, idx1, epi1)

        if cfg.use_collective:
            nc.gpsimd.collective_compute(
                "AllGather", mybir.AluOpType.bypass,
                ins=[cc_all[:]],
                outs=[T2[:]],
                replica_groups=[list(range(NC))],
            )
        else:
            # timing-sim-only stand-in for the AllGather: copy local rows to
            # every rank block of T2 (values wrong cross-core, local DMA cost
            # similar to the receive side of the real collective)
            with tc.tile_pool(name='ccb', bufs=2) as ccbp:
                for cci in range(NT // CCB):
                    bb = ccbp.tile([128, CCB, RB], bf16, tag='bb')
                    nc.sync.dma_start(
                        bb[:], cc_all[cci * CCB * 128:(cci + 1) * CCB * 128,
                                      :].rearrange("(t p) r -> p t r", p=128))
                    for k in range(NC):
                        nc.sync.dma_start(
                            T2[k * SH + cci * CCB * 128:
                               k * SH + (cci + 1) * CCB * 128, :].rearrange(
                                "(t p) r -> p t r", p=128), bb[:])

        # layer-2 dst-attention prefetch: issued after the AllGather so it
        # runs inside its window (reads only the local cc staging)
        prefetch_gd(2, cc_all[0:SH, 0:128], RB, idx2)

        edge_phase(2, T2, idx2, epi2)

    nc.compile()
    return nc


_CACHE = {}


def kernel(x, edge_index, W1, att_src1, att_dst1, b1, W2, att_src2,
           att_dst2, b2):
    cfg = Cfg()
    in_maps, out_node = host_prep(cfg, x, edge_index, W1, att_src1,
                                  att_dst1, b1, W2, att_src2, att_dst2, b2)
    if 'nc' not in _CACHE:
        _CACHE['nc'] = build(cfg)
    nc = _CACHE['nc']
    from concourse.bass_utils import run_bass_kernel_spmd
    res = run_bass_kernel_spmd(nc, in_maps, core_ids=list(range(cfg.ncores)))
    full = np.concatenate([res.results[k]['out2'] for k in range(cfg.ncores)],
                          axis=0)
    flat = out_node.reshape(-1)
    out = np.zeros((cfg.n, cfg.out), np.float32)
    real = flat >= 0
    out[flat[real]] = full[real]
    return out
